# revision 23
# baseline (speedup 1.0000x reference)
"""GQA attention kernel for Trainium2, 8 NeuronCores.

Problem: x[2,2048,2048] @ Wq/Wk/Wv -> grouped-query attention (16 q heads,
4 kv groups, head_dim 128, causal) -> @ Wo + bo.

Sharding: (batch b in 0..1) x (kv group g in 0..3) -> 8 cores.
Each core computes the full attention for its (b, g): 4 query heads sharing
one kv head, then a row-parallel partial of the output projection
(ctx_g @ Wo[g*512:(g+1)*512, :]). Host sums the 4 group partials per batch
and adds the bias.

v2 changes vs baseline (461us):
  - softmax denominator summed on the PE (lhsT=[128,4] one-hot-column ones)
    into a single [4,512] PSUM bank, replacing 160 DVE adds + gpsimd
    partition_all_reduce.
  - reciprocal via DVE reciprocal_approx_fast on [4,512] (was 3.3us/row
    serial InstReciprocal).
  - reciprocal broadcast via PE matmul (lhsT=[4,128] one-hot-row ones),
    replacing gpsimd partition_broadcast.
  - causal diagonal tiles compute only the live column range i >= 128*m;
    affine_select only on the [128,128] triangular strip.
Goal: PE never idles > ~3.4us (stays at 2.4GHz), no DVE/gpsimd critical path.
"""

import os

import ml_dtypes
import numpy as np

import concourse.bass as bass
from concourse import bacc
import concourse.bass_isa as bass_isa
import concourse.mybir as mybir
from concourse.bass_utils import run_bass_kernel_spmd
from concourse.masks import make_identity
from concourse.tile import TileContext

B, N, D = 2, 2048, 2048
G, REP, HD = 4, 4, 128
E = REP * HD  # 512 q-dims per group
P = 128
IB = 512  # i-block (query block) size
NBLK = N // IB  # 4
NCT = D // P  # 16 contraction tiles
NJT = N // P  # 16 key tiles
SCALE = 1.0 / float(np.sqrt(HD))

F32 = mybir.dt.float32
F32R = mybir.dt.float32r
BF16 = mybir.dt.bfloat16

_LAST_RESULT = None  # test.py reads exec_time_ns from here


def _r(ap):
    return ap.bitcast(F32R)


def build_bass():
    nc = bacc.Bacc()
    # All inputs bf16, laid out [partition, chunk, free] on the host so each
    # tensor is a single DMA descriptor (Sync-engine descriptor issue at
    # ~650ns each was the startup bottleneck).
    xT = nc.dram_tensor("xT", [P, NCT, N], BF16, kind="ExternalInput")
    wq = nc.dram_tensor("wq", [P, NCT, E], BF16, kind="ExternalInput")
    wk = nc.dram_tensor("wk", [P, NCT, HD], BF16, kind="ExternalInput")
    wv = nc.dram_tensor("wv", [P, NCT, HD], BF16, kind="ExternalInput")
    wo = nc.dram_tensor("wo", [P, REP, D], BF16, kind="ExternalInput")
    out = nc.dram_tensor("out", [N, D], F32, kind="ExternalOutput")

    with TileContext(nc) as tc:
        build_tile_kernel(nc, tc, xT, wq, wk, wv, wo, out)
    nc.finalize()
    return nc


def build_tile_kernel(nc, tc, xT, wq, wk, wv, wo, out):
    import contextlib

    ctx = contextlib.ExitStack()
    with ctx:
        persist = ctx.enter_context(tc.tile_pool(name="persist", bufs=1))
        weights = ctx.enter_context(tc.tile_pool(name="weights", bufs=1))
        work = ctx.enter_context(tc.tile_pool(name="work", bufs=2))
        psum_mm = ctx.enter_context(
            tc.tile_pool(name="psum_mm", bufs=3, space="PSUM")
        )
        psum_ctx = ctx.enter_context(
            tc.tile_pool(name="psum_ctx", bufs=2, space="PSUM")
        )
        psum_den = ctx.enter_context(
            tc.tile_pool(name="psum_den", bufs=1, space="PSUM")
        )

        # ---- constants ----
        ident = persist.tile([P, P], F32)
        make_identity(nc, ident)
        # sel_ones[r]: [128,4] bf16, column r all ones (den matmul lhsT)
        sel_ones = []
        for r in range(REP):
            t = persist.tile([P, REP], BF16, name=f"selo{r}", tag="selo", bufs=REP)
            nc.vector.memset(t, 0.0)
            nc.vector.memset(t[:, r : r + 1], 1.0)
            sel_ones.append(t)
        # sel4[r]: [4,128] bf16, row r all ones (reciprocal broadcast lhsT).
        # Partition-sliced memset fails BIR verification, so carve the row
        # out of an all-ones tile with affine_select on the channel index.
        sel4 = []
        for r in range(REP):
            t = persist.tile([REP, P], BF16, name=f"sel4{r}", tag="sel4", bufs=REP)
            nc.vector.memset(t, 1.0)
            nc.gpsimd.affine_select(
                out=t,
                in_=t,
                compare_op=mybir.AluOpType.is_equal,
                fill=0.0,
                base=-r,
                pattern=[[0, P]],
                channel_multiplier=1,
            )
            sel4.append(t)

        # ---- weights + block-0 activations in SBUF ----
        # Single-descriptor DMAs, issued in order of first use:
        # wk, xt(block 0), wv, wq, wo.
        wk_all = weights.tile([P, NCT * HD], BF16, name="wk_all", tag="wk")
        nc.sync.dma_start(out=wk_all, in_=wk[:, :, :])
        # block-0 x: 4 descriptors so the transfer spreads across DMA queues
        # (startup is transfer-latency-bound)
        xt0_all = work.tile([P, NCT * IB], BF16, name="xt_all0", tag="xt", bufs=2)
        for q in range(4):
            c0, c1 = q * (NCT // 4), (q + 1) * (NCT // 4)
            nc.sync.dma_start(
                out=xt0_all[:, c0 * IB : c1 * IB], in_=xT[:, c0:c1, 0:IB]
            )
        wv_all = weights.tile([P, NCT * HD], BF16, name="wv_all", tag="wv")
        nc.sync.dma_start(out=wv_all, in_=wv[:, :, :])
        wq_all = weights.tile([P, NCT * E], BF16, name="wq_all", tag="wq")
        nc.sync.dma_start(out=wq_all, in_=wq[:, :, :])
        wo_all = weights.tile([P, REP * D], BF16, name="wo_all", tag="wo")
        nc.sync.dma_start(out=wo_all, in_=wo[:, :, :])

        def wk_sl(ct):
            return wk_all[:, ct * HD : (ct + 1) * HD]

        def wv_sl(ct):
            return wv_all[:, ct * HD : (ct + 1) * HD]

        def wq_sl(ct, r):
            return wq_all[:, ct * E + r * P : ct * E + (r + 1) * P]

        def wo_sl(r, ot):
            return wo_all[:, r * D + ot * IB : r * D + (ot + 1) * IB]

        # persistent activations (full sequence)
        kT = persist.tile([P, N], BF16)  # [d, i]
        v_sb = [
            persist.tile([P, HD], BF16, name=f"v{jt}", tag="v", bufs=NJT)
            for jt in range(NJT)
        ]

        def emit_proj(ib, xt_all=None):
            """Projections for block ib: K, V (natural layout), Q."""
            isl = slice(ib * IB, (ib + 1) * IB)
            if xt_all is None:
                xt_all = work.tile(
                    [P, NCT * IB], BF16, name=f"xt_all{ib}", tag="xt", bufs=2
                )
                nc.sync.dma_start(out=xt_all, in_=xT[:, :, isl])

            def xt_sl(ct):
                return xt_all[:, ct * IB : (ct + 1) * IB]

            ps_k = psum_mm.tile([P, IB], F32, name=f"psk{ib}", tag="mm")
            for ct in range(NCT):
                nc.tensor.matmul(
                    ps_k,
                    lhsT=wk_sl(ct),
                    rhs=xt_sl(ct),
                    start=(ct == 0),
                    stop=(ct == NCT - 1),
                )
            nc.scalar.copy(kT[:, isl], ps_k)

            # V transposed [d, i] in one 512-wide matmul, then 4 PE transposes
            # to natural [j, d]. Transposes borrow the den-pool bank (free
            # between rec4(ib) and the rb broadcasts) and are interleaved into
            # the Q loop so their ACT-copy waits hide behind Q matmuls.
            ps_v = psum_mm.tile([P, IB], F32, name=f"psv{ib}", tag="mm")
            for ct in range(NCT):
                nc.tensor.matmul(
                    ps_v,
                    lhsT=wv_sl(ct),
                    rhs=xt_sl(ct),
                    start=(ct == 0),
                    stop=(ct == NCT - 1),
                )
            vT_b = work.tile([P, IB], F32, name=f"vT{ib}", tag="vT", bufs=2)
            nc.scalar.copy(vT_b, ps_v)

            qT_b = []
            for r in range(REP):
                ps_q = psum_mm.tile([P, IB], F32, name=f"psq{ib}_{r}", tag="mm")
                for ct in range(NCT):
                    nc.tensor.matmul(
                        ps_q,
                        lhsT=wq_sl(ct, r),
                        rhs=xt_sl(ct),
                        start=(ct == 0),
                        stop=(ct == NCT - 1),
                    )
                qt = work.tile([P, IB], BF16, name=f"qT{ib}_{r}", tag="qT", bufs=8)
                nc.scalar.copy(qt, ps_q)
                qT_b.append(qt)
                sub = r
                jt = ib * (IB // P) + sub
                ps_t = psum_den.tile(
                    [P, IB], F32, name=f"pst{jt}", tag="den", bufs=1
                )
                nc.tensor.transpose(
                    ps_t[:, 0:P], vT_b[:, sub * P : (sub + 1) * P], ident
                )
                nc.scalar.copy(v_sb[jt], ps_t[:, 0:P])
            return qT_b

        # Software pipeline: projections for block ib+1 are emitted BETWEEN
        # attention(ib) and the normalize/outproj drain of block ib, so the
        # in-order PE queue fills the drain's dependency bubble with next
        # block's matmuls (keeps the PE HAM un-throttled).
        qT_cur = emit_proj(0, xt0_all)
        for ib in range(NBLK):
            # ============ B: attention for this query block ================
            njt = (ib + 1) * (IB // P)  # causal: key tiles 0..njt-1
            ps_cs = [
                psum_ctx.tile([P, IB], F32, name=f"psc{ib}_{r}", tag=f"ctx{r}", bufs=1)
                for r in range(REP)
            ]
            den_t = psum_den.tile([P, IB], F32, name=f"den{ib}", tag="den", bufs=1)
            den4 = den_t[0:REP, :]
            for jk in range(njt):
                m = jk - ib * (IB // P)
                i0 = max(m, 0) * P  # live columns: i >= 128*m on diagonal
                for r in range(REP):
                    ps_s = psum_mm.tile(
                        [P, IB], F32, name=f"pss{ib}_{r}_{jk}", tag="mm"
                    )
                    nc.tensor.matmul(
                        ps_s[:, i0:],
                        lhsT=kT[:, jk * P : (jk + 1) * P],
                        rhs=qT_cur[r][:, i0:],
                        start=True,
                        stop=True,
                    )
                    ex = work.tile(
                        [P, IB], BF16, name=f"ex{ib}_{r}_{jk}", tag="ex", bufs=6
                    )
                    nc.scalar.activation(
                        ex[:, i0:],
                        ps_s[:, i0:],
                        mybir.ActivationFunctionType.Exp,
                        scale=SCALE,
                    )
                    if m >= 0:
                        # triangular strip: keep where (i - i0) - j >= 0
                        nc.gpsimd.affine_select(
                            out=ex[:, i0 : i0 + P],
                            in_=ex[:, i0 : i0 + P],
                            compare_op=mybir.AluOpType.is_ge,
                            fill=0.0,
                            base=0,
                            pattern=[[1, P]],
                            channel_multiplier=-1,
                        )
                    nc.tensor.matmul(
                        den4[:, i0:],
                        lhsT=sel_ones[r],
                        rhs=ex[:, i0:],
                        start=(jk == 0 and r == 0),
                        stop=(jk == njt - 1 and r == REP - 1),
                        skip_group_check=True,
                    )
                    nc.tensor.matmul(
                        ps_cs[r][:, i0:],
                        lhsT=v_sb[jk],
                        rhs=ex[:, i0:],
                        start=(jk == 0),
                        stop=(jk == njt - 1),
                        skip_group_check=True,
                    )

            # reciprocal on DVE — runs in parallel with next block's proj MMs
            rec4 = work.tile([REP, IB], F32, name=f"rec4{ib}", tag="rec4", bufs=2)
            nc.vector.reciprocal_approx_fast(out=rec4, in_=den4)
            rec4b = work.tile([REP, IB], BF16, name=f"rec4b{ib}", tag="rec4b", bufs=2)
            nc.vector.tensor_copy(rec4b, rec4)

            # ============ A(ib+1): next block's projections ================
            if ib + 1 < NBLK:
                qT_next = emit_proj(ib + 1)
            else:
                qT_next = None

            # ============ C: normalize + output projection =================
            ctxn_b = []
            for r in range(REP):
                ps_rb = psum_den.tile([P, IB], F32, name=f"psrb{ib}_{r}", tag="den", bufs=1)
                nc.tensor.matmul(
                    ps_rb,
                    lhsT=sel4[r],
                    rhs=rec4b,
                    start=True,
                    stop=True,
                )
                rb_sb = work.tile([P, IB], BF16, name=f"rb{ib}_{r}", tag="rb", bufs=2)
                nc.scalar.copy(rb_sb, ps_rb)
                cn = work.tile([P, IB], BF16, name=f"cn{ib}_{r}", tag="ctxn", bufs=8)
                nc.vector.tensor_mul(cn, ps_cs[r], rb_sb)
                ctxn_b.append(cn)

            for sub in range(IB // P):
                it = ib * (IB // P) + sub
                ssl = slice(sub * P, (sub + 1) * P)
                o_sb = work.tile([P, D], F32, name=f"osb{it}", tag="osb", bufs=2)
                for ot in range(D // IB):
                    ps_o = psum_mm.tile([P, IB], F32, name=f"pso{it}_{ot}", tag="mm")
                    for r in range(REP):
                        nc.tensor.matmul(
                            ps_o,
                            lhsT=ctxn_b[r][:, ssl],
                            rhs=wo_sl(r, ot),
                            start=(r == 0),
                            stop=(r == REP - 1),
                        )
                    nc.vector.tensor_copy(o_sb[:, ot * IB : (ot + 1) * IB], ps_o)
                for q in range(2):
                    hsl = slice(q * (D // 2), (q + 1) * (D // 2))
                    nc.sync.dma_start(
                        out=out[it * P : (it + 1) * P, hsl], in_=o_sb[:, hsl]
                    )
            qT_cur = qT_next


_NC_CACHE = None


def kernel(x, Wq, Wk, Wv, Wo, bo):
    global _LAST_RESULT, _NC_CACHE
    x = np.asarray(x, dtype=np.float32)
    Wq = np.asarray(Wq, dtype=np.float32)
    Wk = np.asarray(Wk, dtype=np.float32)
    Wv = np.asarray(Wv, dtype=np.float32)
    Wo = np.asarray(Wo, dtype=np.float32)
    bo = np.asarray(bo, dtype=np.float32)

    if _NC_CACHE is None:
        _NC_CACHE = build_bass()
    nc = _NC_CACHE

    def chunked(a, pdim):
        # [pdim*nchunk, F] -> [pdim, nchunk, F] bf16, partition-major
        nchunk = a.shape[0] // pdim
        return np.ascontiguousarray(
            a.reshape(nchunk, pdim, a.shape[1]).transpose(1, 0, 2)
        ).astype(ml_dtypes.bfloat16)

    in_maps = []
    for core in range(8):
        b, g = core // G, core % G
        in_maps.append(
            {
                "xT": chunked(np.ascontiguousarray(x[b].T), P),
                "wq": chunked(Wq[:, g * E : (g + 1) * E], P),
                "wk": chunked(Wk[:, g * HD : (g + 1) * HD], P),
                "wv": chunked(Wv[:, g * HD : (g + 1) * HD], P),
                "wo": chunked(Wo[g * E : (g + 1) * E, :], P),
            }
        )
    res = run_bass_kernel_spmd(
        nc,
        in_maps,
        list(range(8)),
        trace=bool(os.environ.get("BASS_TRACE")),
    )
    _LAST_RESULT = res
    partials = np.stack(
        [np.asarray(r["out"]).astype(np.float32) for r in res.results]
    )  # [8, N, D]
    full = partials.reshape(B, G, N, D).sum(axis=1) + bo[None, None, :]
    return full.astype(np.float32)


# revision 24
# speedup vs baseline: 1.0114x; 1.0114x over previous
"""GQA attention kernel for Trainium2, 8 NeuronCores.

Problem: x[2,2048,2048] @ Wq/Wk/Wv -> grouped-query attention (16 q heads,
4 kv groups, head_dim 128, causal) -> @ Wo + bo.

Sharding: (batch b in 0..1) x (kv group g in 0..3) -> 8 cores.
Each core computes the full attention for its (b, g): 4 query heads sharing
one kv head, then a row-parallel partial of the output projection
(ctx_g @ Wo[g*512:(g+1)*512, :]). Host sums the 4 group partials per batch
and adds the bias.

v2 changes vs baseline (461us):
  - softmax denominator summed on the PE (lhsT=[128,4] one-hot-column ones)
    into a single [4,512] PSUM bank, replacing 160 DVE adds + gpsimd
    partition_all_reduce.
  - reciprocal via DVE reciprocal_approx_fast on [4,512] (was 3.3us/row
    serial InstReciprocal).
  - reciprocal broadcast via PE matmul (lhsT=[4,128] one-hot-row ones),
    replacing gpsimd partition_broadcast.
  - causal diagonal tiles compute only the live column range i >= 128*m;
    affine_select only on the [128,128] triangular strip.
Goal: PE never idles > ~3.4us (stays at 2.4GHz), no DVE/gpsimd critical path.
"""

import os

import ml_dtypes
import numpy as np

import concourse.bass as bass
from concourse import bacc
import concourse.bass_isa as bass_isa
import concourse.mybir as mybir
from concourse.bass_utils import run_bass_kernel_spmd
from concourse.masks import make_identity
from concourse.tile import TileContext

B, N, D = 2, 2048, 2048
G, REP, HD = 4, 4, 128
E = REP * HD  # 512 q-dims per group
P = 128
IB = 512  # i-block (query block) size
NBLK = N // IB  # 4
NCT = D // P  # 16 contraction tiles
NJT = N // P  # 16 key tiles
SCALE = 1.0 / float(np.sqrt(HD))

F32 = mybir.dt.float32
F32R = mybir.dt.float32r
BF16 = mybir.dt.bfloat16

_LAST_RESULT = None  # test.py reads exec_time_ns from here


def _r(ap):
    return ap.bitcast(F32R)


def build_bass():
    nc = bacc.Bacc()
    # All inputs bf16, laid out [partition, chunk, free] on the host so each
    # tensor is a single DMA descriptor (Sync-engine descriptor issue at
    # ~650ns each was the startup bottleneck).
    xT = nc.dram_tensor("xT", [P, NCT, N], BF16, kind="ExternalInput")
    wq = nc.dram_tensor("wq", [P, NCT, E], BF16, kind="ExternalInput")
    wk = nc.dram_tensor("wk", [P, NCT, HD], BF16, kind="ExternalInput")
    wv = nc.dram_tensor("wv", [P, NCT, HD], BF16, kind="ExternalInput")
    wo = nc.dram_tensor("wo", [P, REP, D], BF16, kind="ExternalInput")
    out = nc.dram_tensor("out", [N, D], F32, kind="ExternalOutput")

    with TileContext(nc) as tc:
        build_tile_kernel(nc, tc, xT, wq, wk, wv, wo, out)
    nc.finalize()
    return nc


def build_tile_kernel(nc, tc, xT, wq, wk, wv, wo, out):
    import contextlib

    ctx = contextlib.ExitStack()
    with ctx:
        persist = ctx.enter_context(tc.tile_pool(name="persist", bufs=1))
        weights = ctx.enter_context(tc.tile_pool(name="weights", bufs=1))
        work = ctx.enter_context(tc.tile_pool(name="work", bufs=2))
        psum_mm = ctx.enter_context(
            tc.tile_pool(name="psum_mm", bufs=3, space="PSUM")
        )
        psum_ctx = ctx.enter_context(
            tc.tile_pool(name="psum_ctx", bufs=2, space="PSUM")
        )
        psum_den = ctx.enter_context(
            tc.tile_pool(name="psum_den", bufs=1, space="PSUM")
        )

        # ---- constants ----
        ident = persist.tile([P, P], F32)
        make_identity(nc, ident)
        # sel_ones[r]: [128,4] bf16, column r all ones (den matmul lhsT)
        sel_ones = []
        for r in range(REP):
            t = persist.tile([P, REP], BF16, name=f"selo{r}", tag="selo", bufs=REP)
            nc.vector.memset(t, 0.0)
            nc.vector.memset(t[:, r : r + 1], 1.0)
            sel_ones.append(t)
        # sel4[r]: [4,128] bf16, row r all ones (reciprocal broadcast lhsT).
        # Partition-sliced memset fails BIR verification, so carve the row
        # out of an all-ones tile with affine_select on the channel index.
        sel4 = []
        for r in range(REP):
            t = persist.tile([REP, P], BF16, name=f"sel4{r}", tag="sel4", bufs=REP)
            nc.vector.memset(t, 1.0)
            nc.gpsimd.affine_select(
                out=t,
                in_=t,
                compare_op=mybir.AluOpType.is_equal,
                fill=0.0,
                base=-r,
                pattern=[[0, P]],
                channel_multiplier=1,
            )
            sel4.append(t)

        # ---- weights + block-0 activations in SBUF ----
        # Single-descriptor DMAs, issued in order of first use:
        # wk, xt(block 0), wv, wq, wo.
        wk_all = weights.tile([P, NCT * HD], BF16, name="wk_all", tag="wk")
        nc.sync.dma_start(out=wk_all, in_=wk[:, :, :])
        # block-0 x: 4 descriptors so the transfer spreads across DMA queues
        # (startup is transfer-latency-bound)
        xt0_all = work.tile([P, NCT * IB], BF16, name="xt_all0", tag="xt", bufs=2)
        for q in range(4):
            c0, c1 = q * (NCT // 4), (q + 1) * (NCT // 4)
            nc.sync.dma_start(
                out=xt0_all[:, c0 * IB : c1 * IB], in_=xT[:, c0:c1, 0:IB]
            )
        wv_all = weights.tile([P, NCT * HD], BF16, name="wv_all", tag="wv")
        nc.sync.dma_start(out=wv_all, in_=wv[:, :, :])
        wq_all = weights.tile([P, NCT * E], BF16, name="wq_all", tag="wq")
        nc.sync.dma_start(out=wq_all, in_=wq[:, :, :])
        wo_all = weights.tile([P, REP * D], BF16, name="wo_all", tag="wo")
        nc.sync.dma_start(out=wo_all, in_=wo[:, :, :])

        def wk_sl(ct):
            return wk_all[:, ct * HD : (ct + 1) * HD]

        def wv_sl(ct):
            return wv_all[:, ct * HD : (ct + 1) * HD]

        def wq_sl(ct, r):
            return wq_all[:, ct * E + r * P : ct * E + (r + 1) * P]

        def wo_sl(r, ot):
            return wo_all[:, r * D + ot * IB : r * D + (ot + 1) * IB]

        # persistent activations (full sequence)
        kT = persist.tile([P, N], BF16)  # [d, i]
        v_sb = [
            persist.tile([P, HD], BF16, name=f"v{jt}", tag="v", bufs=NJT)
            for jt in range(NJT)
        ]

        def emit_proj(ib, xt_all=None):
            """Projections for block ib: K, V (natural layout), Q."""
            isl = slice(ib * IB, (ib + 1) * IB)
            if xt_all is None:
                xt_all = work.tile(
                    [P, NCT * IB], BF16, name=f"xt_all{ib}", tag="xt", bufs=2
                )
                nc.sync.dma_start(out=xt_all, in_=xT[:, :, isl])

            def xt_sl(ct):
                return xt_all[:, ct * IB : (ct + 1) * IB]

            ps_k = psum_mm.tile([P, IB], F32, name=f"psk{ib}", tag="mm")
            for ct in range(NCT):
                nc.tensor.matmul(
                    ps_k,
                    lhsT=wk_sl(ct),
                    rhs=xt_sl(ct),
                    start=(ct == 0),
                    stop=(ct == NCT - 1),
                )
            nc.scalar.copy(kT[:, isl], ps_k)

            # V transposed [d, i] in one 512-wide matmul, then 4 PE transposes
            # to natural [j, d]. Transposes borrow the den-pool bank (free
            # between rec4(ib) and the rb broadcasts) and are interleaved into
            # the Q loop so their ACT-copy waits hide behind Q matmuls.
            ps_v = psum_mm.tile([P, IB], F32, name=f"psv{ib}", tag="mm")
            for ct in range(NCT):
                nc.tensor.matmul(
                    ps_v,
                    lhsT=wv_sl(ct),
                    rhs=xt_sl(ct),
                    start=(ct == 0),
                    stop=(ct == NCT - 1),
                )
            vT_b = work.tile([P, IB], F32, name=f"vT{ib}", tag="vT", bufs=2)
            nc.scalar.copy(vT_b, ps_v)

            qT_b = []
            for r in range(REP):
                ps_q = psum_mm.tile([P, IB], F32, name=f"psq{ib}_{r}", tag="mm")
                for ct in range(NCT):
                    nc.tensor.matmul(
                        ps_q,
                        lhsT=wq_sl(ct, r),
                        rhs=xt_sl(ct),
                        start=(ct == 0),
                        stop=(ct == NCT - 1),
                    )
                qt = work.tile([P, IB], BF16, name=f"qT{ib}_{r}", tag="qT", bufs=8)
                nc.scalar.copy(qt, ps_q)
                qT_b.append(qt)
                sub = r
                jt = ib * (IB // P) + sub
                ps_t = psum_mm.tile([P, IB], F32, name=f"pst{jt}", tag="mm")
                nc.tensor.transpose(
                    ps_t[:, 0:P], vT_b[:, sub * P : (sub + 1) * P], ident
                )
                nc.scalar.copy(v_sb[jt], ps_t[:, 0:P])
            return qT_b

        # Software pipeline: projections for block ib+1 are emitted BETWEEN
        # attention(ib) and the normalize/outproj drain of block ib, so the
        # in-order PE queue fills the drain's dependency bubble with next
        # block's matmuls (keeps the PE HAM un-throttled).
        qT_cur = emit_proj(0, xt0_all)
        for ib in range(NBLK):
            # ============ B: attention for this query block ================
            njt = (ib + 1) * (IB // P)  # causal: key tiles 0..njt-1
            ps_cs = [
                psum_ctx.tile([P, IB], F32, name=f"psc{ib}_{r}", tag=f"ctx{r}", bufs=1)
                for r in range(REP)
            ]
            den_t = psum_den.tile([P, IB], F32, name=f"den{ib}", tag="den", bufs=1)
            den4 = den_t[0:REP, :]
            for jk in range(njt):
                m = jk - ib * (IB // P)
                i0 = max(m, 0) * P  # live columns: i >= 128*m on diagonal
                for r in range(REP):
                    ps_s = psum_mm.tile(
                        [P, IB], F32, name=f"pss{ib}_{r}_{jk}", tag="mm"
                    )
                    nc.tensor.matmul(
                        ps_s[:, i0:],
                        lhsT=kT[:, jk * P : (jk + 1) * P],
                        rhs=qT_cur[r][:, i0:],
                        start=True,
                        stop=True,
                    )
                    ex = work.tile(
                        [P, IB], BF16, name=f"ex{ib}_{r}_{jk}", tag="ex", bufs=6
                    )
                    nc.scalar.activation(
                        ex[:, i0:],
                        ps_s[:, i0:],
                        mybir.ActivationFunctionType.Exp,
                        scale=SCALE,
                    )
                    if m >= 0:
                        # triangular strip: keep where (i - i0) - j >= 0
                        nc.gpsimd.affine_select(
                            out=ex[:, i0 : i0 + P],
                            in_=ex[:, i0 : i0 + P],
                            compare_op=mybir.AluOpType.is_ge,
                            fill=0.0,
                            base=0,
                            pattern=[[1, P]],
                            channel_multiplier=-1,
                        )
                    nc.tensor.matmul(
                        den4[:, i0:],
                        lhsT=sel_ones[r],
                        rhs=ex[:, i0:],
                        start=(jk == 0 and r == 0),
                        stop=(jk == njt - 1 and r == REP - 1),
                        skip_group_check=True,
                    )
                    nc.tensor.matmul(
                        ps_cs[r][:, i0:],
                        lhsT=v_sb[jk],
                        rhs=ex[:, i0:],
                        start=(jk == 0),
                        stop=(jk == njt - 1),
                        skip_group_check=True,
                    )

            # reciprocal on DVE — runs in parallel with next block's proj MMs
            rec4 = work.tile([REP, IB], F32, name=f"rec4{ib}", tag="rec4", bufs=2)
            nc.vector.reciprocal_approx_fast(out=rec4, in_=den4)
            rec4b = work.tile([REP, IB], BF16, name=f"rec4b{ib}", tag="rec4b", bufs=2)
            nc.vector.tensor_copy(rec4b, rec4)

            # ============ A(ib+1): next block's projections ================
            if ib + 1 < NBLK:
                qT_next = emit_proj(ib + 1)
            else:
                qT_next = None

            # ============ C: normalize + output projection =================
            ctxn_b = []
            for r in range(REP):
                ps_rb = psum_den.tile([P, IB], F32, name=f"psrb{ib}_{r}", tag="den", bufs=1)
                nc.tensor.matmul(
                    ps_rb,
                    lhsT=sel4[r],
                    rhs=rec4b,
                    start=True,
                    stop=True,
                )
                rb_sb = work.tile([P, IB], BF16, name=f"rb{ib}_{r}", tag="rb", bufs=2)
                nc.scalar.copy(rb_sb, ps_rb)
                cn = work.tile([P, IB], BF16, name=f"cn{ib}_{r}", tag="ctxn", bufs=8)
                nc.vector.tensor_mul(cn, ps_cs[r], rb_sb)
                ctxn_b.append(cn)

            for sub in range(IB // P):
                it = ib * (IB // P) + sub
                ssl = slice(sub * P, (sub + 1) * P)
                o_sb = work.tile([P, D], F32, name=f"osb{it}", tag="osb", bufs=2)
                for ot in range(D // IB):
                    ps_o = psum_mm.tile([P, IB], F32, name=f"pso{it}_{ot}", tag="mm")
                    for r in range(REP):
                        nc.tensor.matmul(
                            ps_o,
                            lhsT=ctxn_b[r][:, ssl],
                            rhs=wo_sl(r, ot),
                            start=(r == 0),
                            stop=(r == REP - 1),
                        )
                    nc.vector.tensor_copy(o_sb[:, ot * IB : (ot + 1) * IB], ps_o)
                for q in range(2):
                    hsl = slice(q * (D // 2), (q + 1) * (D // 2))
                    nc.sync.dma_start(
                        out=out[it * P : (it + 1) * P, hsl], in_=o_sb[:, hsl]
                    )
            qT_cur = qT_next


_NC_CACHE = None


def kernel(x, Wq, Wk, Wv, Wo, bo):
    global _LAST_RESULT, _NC_CACHE
    x = np.asarray(x, dtype=np.float32)
    Wq = np.asarray(Wq, dtype=np.float32)
    Wk = np.asarray(Wk, dtype=np.float32)
    Wv = np.asarray(Wv, dtype=np.float32)
    Wo = np.asarray(Wo, dtype=np.float32)
    bo = np.asarray(bo, dtype=np.float32)

    if _NC_CACHE is None:
        _NC_CACHE = build_bass()
    nc = _NC_CACHE

    def chunked(a, pdim):
        # [pdim*nchunk, F] -> [pdim, nchunk, F] bf16, partition-major
        nchunk = a.shape[0] // pdim
        return np.ascontiguousarray(
            a.reshape(nchunk, pdim, a.shape[1]).transpose(1, 0, 2)
        ).astype(ml_dtypes.bfloat16)

    in_maps = []
    for core in range(8):
        b, g = core // G, core % G
        in_maps.append(
            {
                "xT": chunked(np.ascontiguousarray(x[b].T), P),
                "wq": chunked(Wq[:, g * E : (g + 1) * E], P),
                "wk": chunked(Wk[:, g * HD : (g + 1) * HD], P),
                "wv": chunked(Wv[:, g * HD : (g + 1) * HD], P),
                "wo": chunked(Wo[g * E : (g + 1) * E, :], P),
            }
        )
    res = run_bass_kernel_spmd(
        nc,
        in_maps,
        list(range(8)),
        trace=bool(os.environ.get("BASS_TRACE")),
    )
    _LAST_RESULT = res
    partials = np.stack(
        [np.asarray(r["out"]).astype(np.float32) for r in res.results]
    )  # [8, N, D]
    full = partials.reshape(B, G, N, D).sum(axis=1) + bo[None, None, :]
    return full.astype(np.float32)


# revision 25
# speedup vs baseline: 1.0215x; 1.0100x over previous
"""GQA attention kernel for Trainium2, 8 NeuronCores.

Problem: x[2,2048,2048] @ Wq/Wk/Wv -> grouped-query attention (16 q heads,
4 kv groups, head_dim 128, causal) -> @ Wo + bo.

Sharding: (batch b in 0..1) x (kv group g in 0..3) -> 8 cores.
Each core computes the full attention for its (b, g): 4 query heads sharing
one kv head, then a row-parallel partial of the output projection
(ctx_g @ Wo[g*512:(g+1)*512, :]). Host sums the 4 group partials per batch
and adds the bias.

v2 changes vs baseline (461us):
  - softmax denominator summed on the PE (lhsT=[128,4] one-hot-column ones)
    into a single [4,512] PSUM bank, replacing 160 DVE adds + gpsimd
    partition_all_reduce.
  - reciprocal via DVE reciprocal_approx_fast on [4,512] (was 3.3us/row
    serial InstReciprocal).
  - reciprocal broadcast via PE matmul (lhsT=[4,128] one-hot-row ones),
    replacing gpsimd partition_broadcast.
  - causal diagonal tiles compute only the live column range i >= 128*m;
    affine_select only on the [128,128] triangular strip.
Goal: PE never idles > ~3.4us (stays at 2.4GHz), no DVE/gpsimd critical path.
"""

import os

import ml_dtypes
import numpy as np

import concourse.bass as bass
from concourse import bacc
import concourse.bass_isa as bass_isa
import concourse.mybir as mybir
from concourse.bass_utils import run_bass_kernel_spmd
from concourse.masks import make_identity
from concourse.tile import TileContext

B, N, D = 2, 2048, 2048
G, REP, HD = 4, 4, 128
E = REP * HD  # 512 q-dims per group
P = 128
IB = 512  # i-block (query block) size
NBLK = N // IB  # 4
NCT = D // P  # 16 contraction tiles
NJT = N // P  # 16 key tiles
SCALE = 1.0 / float(np.sqrt(HD))

F32 = mybir.dt.float32
F32R = mybir.dt.float32r
BF16 = mybir.dt.bfloat16

_LAST_RESULT = None  # test.py reads exec_time_ns from here


def _r(ap):
    return ap.bitcast(F32R)


def build_bass():
    nc = bacc.Bacc()
    # All inputs bf16, laid out [partition, chunk, free] on the host so each
    # tensor is a single DMA descriptor (Sync-engine descriptor issue at
    # ~650ns each was the startup bottleneck).
    xT = nc.dram_tensor("xT", [P, NCT, N], BF16, kind="ExternalInput")
    wq = nc.dram_tensor("wq", [P, NCT, E], BF16, kind="ExternalInput")
    wk = nc.dram_tensor("wk", [P, NCT, HD], BF16, kind="ExternalInput")
    wv = nc.dram_tensor("wv", [P, NCT, HD], BF16, kind="ExternalInput")
    wo = nc.dram_tensor("wo", [P, REP, D], BF16, kind="ExternalInput")
    out = nc.dram_tensor("out", [N, D], F32, kind="ExternalOutput")

    with TileContext(nc) as tc:
        build_tile_kernel(nc, tc, xT, wq, wk, wv, wo, out)
    nc.finalize()
    return nc


def build_tile_kernel(nc, tc, xT, wq, wk, wv, wo, out):
    import contextlib

    ctx = contextlib.ExitStack()
    with ctx:
        persist = ctx.enter_context(tc.tile_pool(name="persist", bufs=1))
        weights = ctx.enter_context(tc.tile_pool(name="weights", bufs=1))
        work = ctx.enter_context(tc.tile_pool(name="work", bufs=2))
        psum_mm = ctx.enter_context(
            tc.tile_pool(name="psum_mm", bufs=3, space="PSUM")
        )
        psum_ctx = ctx.enter_context(
            tc.tile_pool(name="psum_ctx", bufs=2, space="PSUM")
        )
        psum_den = ctx.enter_context(
            tc.tile_pool(name="psum_den", bufs=1, space="PSUM")
        )

        # ---- constants ----
        ident = persist.tile([P, P], F32)
        make_identity(nc, ident)
        # sel_ones[r]: [128,4] bf16, column r all ones (den matmul lhsT)
        sel_ones = []
        for r in range(REP):
            t = persist.tile([P, REP], BF16, name=f"selo{r}", tag="selo", bufs=REP)
            nc.vector.memset(t, 0.0)
            nc.vector.memset(t[:, r : r + 1], 1.0)
            sel_ones.append(t)
        # sel4[r]: [4,128] bf16, row r all ones (reciprocal broadcast lhsT).
        # Partition-sliced memset fails BIR verification, so carve the row
        # out of an all-ones tile with affine_select on the channel index.
        sel4 = []
        for r in range(REP):
            t = persist.tile([REP, P], BF16, name=f"sel4{r}", tag="sel4", bufs=REP)
            nc.vector.memset(t, 1.0)
            nc.gpsimd.affine_select(
                out=t,
                in_=t,
                compare_op=mybir.AluOpType.is_equal,
                fill=0.0,
                base=-r,
                pattern=[[0, P]],
                channel_multiplier=1,
            )
            sel4.append(t)

        # ---- weights + block-0 activations in SBUF ----
        # Single-descriptor DMAs, issued in order of first use:
        # wk, xt(block 0), wv, wq, wo.
        wk_all = weights.tile([P, NCT * HD], BF16, name="wk_all", tag="wk")
        nc.sync.dma_start(out=wk_all, in_=wk[:, :, :])
        # block-0 x: 4 descriptors so the transfer spreads across DMA queues
        # (startup is transfer-latency-bound)
        xt0_all = work.tile([P, NCT * IB], BF16, name="xt_all0", tag="xt", bufs=2)
        for q in range(4):
            c0, c1 = q * (NCT // 4), (q + 1) * (NCT // 4)
            nc.sync.dma_start(
                out=xt0_all[:, c0 * IB : c1 * IB], in_=xT[:, c0:c1, 0:IB]
            )
        wv_all = weights.tile([P, NCT * HD], BF16, name="wv_all", tag="wv")
        nc.sync.dma_start(out=wv_all, in_=wv[:, :, :])
        wq_all = weights.tile([P, NCT * E], BF16, name="wq_all", tag="wq")
        nc.sync.dma_start(out=wq_all, in_=wq[:, :, :])
        wo_all = weights.tile([P, REP * D], BF16, name="wo_all", tag="wo")
        nc.sync.dma_start(out=wo_all, in_=wo[:, :, :])

        def wk_sl(ct):
            return wk_all[:, ct * HD : (ct + 1) * HD]

        def wv_sl(ct):
            return wv_all[:, ct * HD : (ct + 1) * HD]

        def wq_sl(ct, r):
            return wq_all[:, ct * E + r * P : ct * E + (r + 1) * P]

        def wo_sl(r, ot):
            return wo_all[:, r * D + ot * IB : r * D + (ot + 1) * IB]

        # persistent activations (full sequence)
        kT = persist.tile([P, N], BF16)  # [d, i]
        v_sb = [
            persist.tile([P, HD], BF16, name=f"v{jt}", tag="v", bufs=NJT)
            for jt in range(NJT)
        ]

        def emit_proj(ib, xt_all=None):
            """Projections for block ib: K, V (natural layout), Q."""
            isl = slice(ib * IB, (ib + 1) * IB)
            if xt_all is None:
                xt_all = work.tile(
                    [P, NCT * IB], BF16, name=f"xt_all{ib}", tag="xt", bufs=2
                )
                nc.sync.dma_start(out=xt_all, in_=xT[:, :, isl])

            def xt_sl(ct):
                return xt_all[:, ct * IB : (ct + 1) * IB]

            ps_k = psum_mm.tile([P, IB], F32, name=f"psk{ib}", tag="mm")
            for ct in range(NCT):
                nc.tensor.matmul(
                    ps_k,
                    lhsT=wk_sl(ct),
                    rhs=xt_sl(ct),
                    start=(ct == 0),
                    stop=(ct == NCT - 1),
                )
            nc.scalar.copy(kT[:, isl], ps_k)

            # V directly in natural [j, d] layout: lhsT = a 128-query strip of
            # xT (contraction c on partitions), rhs = Wv tile. No transpose.
            for sub in range(IB // P):
                jt = ib * (IB // P) + sub
                ps_v = psum_mm.tile([P, IB], F32, name=f"psv{jt}", tag="mm")
                for ct in range(NCT):
                    nc.tensor.matmul(
                        ps_v[:, 0:HD],
                        lhsT=xt_all[
                            :, ct * IB + sub * P : ct * IB + (sub + 1) * P
                        ],
                        rhs=wv_sl(ct),
                        start=(ct == 0),
                        stop=(ct == NCT - 1),
                    )
                nc.scalar.copy(v_sb[jt], ps_v[:, 0:HD])

            qT_b = []
            for r in range(REP):
                ps_q = psum_mm.tile([P, IB], F32, name=f"psq{ib}_{r}", tag="mm")
                for ct in range(NCT):
                    nc.tensor.matmul(
                        ps_q,
                        lhsT=wq_sl(ct, r),
                        rhs=xt_sl(ct),
                        start=(ct == 0),
                        stop=(ct == NCT - 1),
                    )
                qt = work.tile([P, IB], BF16, name=f"qT{ib}_{r}", tag="qT", bufs=8)
                nc.scalar.copy(qt, ps_q)
                qT_b.append(qt)
            return qT_b

        # Software pipeline: projections for block ib+1 are emitted BETWEEN
        # attention(ib) and the normalize/outproj drain of block ib, so the
        # in-order PE queue fills the drain's dependency bubble with next
        # block's matmuls (keeps the PE HAM un-throttled).
        qT_cur = emit_proj(0, xt0_all)
        for ib in range(NBLK):
            # ============ B: attention for this query block ================
            njt = (ib + 1) * (IB // P)  # causal: key tiles 0..njt-1
            ps_cs = [
                psum_ctx.tile([P, IB], F32, name=f"psc{ib}_{r}", tag=f"ctx{r}", bufs=1)
                for r in range(REP)
            ]
            den_t = psum_den.tile([P, IB], F32, name=f"den{ib}", tag="den", bufs=1)
            den4 = den_t[0:REP, :]
            for jk in range(njt):
                m = jk - ib * (IB // P)
                i0 = max(m, 0) * P  # live columns: i >= 128*m on diagonal
                for r in range(REP):
                    ps_s = psum_mm.tile(
                        [P, IB], F32, name=f"pss{ib}_{r}_{jk}", tag="mm"
                    )
                    nc.tensor.matmul(
                        ps_s[:, i0:],
                        lhsT=kT[:, jk * P : (jk + 1) * P],
                        rhs=qT_cur[r][:, i0:],
                        start=True,
                        stop=True,
                    )
                    ex = work.tile(
                        [P, IB], BF16, name=f"ex{ib}_{r}_{jk}", tag="ex", bufs=6
                    )
                    nc.scalar.activation(
                        ex[:, i0:],
                        ps_s[:, i0:],
                        mybir.ActivationFunctionType.Exp,
                        scale=SCALE,
                    )
                    if m >= 0:
                        # triangular strip: keep where (i - i0) - j >= 0
                        nc.gpsimd.affine_select(
                            out=ex[:, i0 : i0 + P],
                            in_=ex[:, i0 : i0 + P],
                            compare_op=mybir.AluOpType.is_ge,
                            fill=0.0,
                            base=0,
                            pattern=[[1, P]],
                            channel_multiplier=-1,
                        )
                    nc.tensor.matmul(
                        den4[:, i0:],
                        lhsT=sel_ones[r],
                        rhs=ex[:, i0:],
                        start=(jk == 0 and r == 0),
                        stop=(jk == njt - 1 and r == REP - 1),
                        skip_group_check=True,
                    )
                    nc.tensor.matmul(
                        ps_cs[r][:, i0:],
                        lhsT=v_sb[jk],
                        rhs=ex[:, i0:],
                        start=(jk == 0),
                        stop=(jk == njt - 1),
                        skip_group_check=True,
                    )

            # reciprocal on DVE — runs in parallel with next block's proj MMs
            rec4 = work.tile([REP, IB], F32, name=f"rec4{ib}", tag="rec4", bufs=2)
            nc.vector.reciprocal_approx_fast(out=rec4, in_=den4)
            rec4b = work.tile([REP, IB], BF16, name=f"rec4b{ib}", tag="rec4b", bufs=2)
            nc.vector.tensor_copy(rec4b, rec4)

            # ============ A(ib+1): next block's projections ================
            if ib + 1 < NBLK:
                qT_next = emit_proj(ib + 1)
            else:
                qT_next = None

            # ============ C: normalize + output projection =================
            ctxn_b = []
            for r in range(REP):
                ps_rb = psum_den.tile([P, IB], F32, name=f"psrb{ib}_{r}", tag="den", bufs=1)
                nc.tensor.matmul(
                    ps_rb,
                    lhsT=sel4[r],
                    rhs=rec4b,
                    start=True,
                    stop=True,
                )
                rb_sb = work.tile([P, IB], BF16, name=f"rb{ib}_{r}", tag="rb", bufs=2)
                nc.scalar.copy(rb_sb, ps_rb)
                cn = work.tile([P, IB], BF16, name=f"cn{ib}_{r}", tag="ctxn", bufs=8)
                nc.vector.tensor_mul(cn, ps_cs[r], rb_sb)
                ctxn_b.append(cn)

            for sub in range(IB // P):
                it = ib * (IB // P) + sub
                ssl = slice(sub * P, (sub + 1) * P)
                o_sb = work.tile([P, D], F32, name=f"osb{it}", tag="osb", bufs=2)
                for ot in range(D // IB):
                    ps_o = psum_mm.tile([P, IB], F32, name=f"pso{it}_{ot}", tag="mm")
                    for r in range(REP):
                        nc.tensor.matmul(
                            ps_o,
                            lhsT=ctxn_b[r][:, ssl],
                            rhs=wo_sl(r, ot),
                            start=(r == 0),
                            stop=(r == REP - 1),
                        )
                    nc.vector.tensor_copy(o_sb[:, ot * IB : (ot + 1) * IB], ps_o)
                for q in range(2):
                    hsl = slice(q * (D // 2), (q + 1) * (D // 2))
                    nc.sync.dma_start(
                        out=out[it * P : (it + 1) * P, hsl], in_=o_sb[:, hsl]
                    )
            qT_cur = qT_next


_NC_CACHE = None


def kernel(x, Wq, Wk, Wv, Wo, bo):
    global _LAST_RESULT, _NC_CACHE
    x = np.asarray(x, dtype=np.float32)
    Wq = np.asarray(Wq, dtype=np.float32)
    Wk = np.asarray(Wk, dtype=np.float32)
    Wv = np.asarray(Wv, dtype=np.float32)
    Wo = np.asarray(Wo, dtype=np.float32)
    bo = np.asarray(bo, dtype=np.float32)

    if _NC_CACHE is None:
        _NC_CACHE = build_bass()
    nc = _NC_CACHE

    def chunked(a, pdim):
        # [pdim*nchunk, F] -> [pdim, nchunk, F] bf16, partition-major
        nchunk = a.shape[0] // pdim
        return np.ascontiguousarray(
            a.reshape(nchunk, pdim, a.shape[1]).transpose(1, 0, 2)
        ).astype(ml_dtypes.bfloat16)

    in_maps = []
    for core in range(8):
        b, g = core // G, core % G
        in_maps.append(
            {
                "xT": chunked(np.ascontiguousarray(x[b].T), P),
                "wq": chunked(Wq[:, g * E : (g + 1) * E], P),
                "wk": chunked(Wk[:, g * HD : (g + 1) * HD], P),
                "wv": chunked(Wv[:, g * HD : (g + 1) * HD], P),
                "wo": chunked(Wo[g * E : (g + 1) * E, :], P),
            }
        )
    res = run_bass_kernel_spmd(
        nc,
        in_maps,
        list(range(8)),
        trace=bool(os.environ.get("BASS_TRACE")),
    )
    _LAST_RESULT = res
    partials = np.stack(
        [np.asarray(r["out"]).astype(np.float32) for r in res.results]
    )  # [8, N, D]
    full = partials.reshape(B, G, N, D).sum(axis=1) + bo[None, None, :]
    return full.astype(np.float32)
